# revision 8
# baseline (speedup 1.0000x reference)
"""Multi-head graph-attention layer for Trainium2 (8-core SPMD).

The reference computes per-head projections hp = einsum("bnf,hfd->bhnd", h, W),
dense attention scores e = hp @ hp^T, LeakyReLU, softmax over the last axis,
and then multiplies hp by sum_j(softmax(e))_j. The sum of a softmax over its
own normalization axis is identically 1, so the layer's exact mathematical
output is hp itself (concatenated over heads):

    out[b, n, h*64+d] = sum_f h[b,n,f] * W[h,f,d]  =  (h[b] @ Wc)[n, h*64+d]

with Wc[f, h*64+d] = W[h,f,d]. The reference's deviation from rowsum==1 is
fp32 rounding noise (~1e-6 relative) that no reimplementation reproduces, so
computing the projection directly is both the fastest and the most accurate
realization. `adj` is unused by the reference and is ignored here.

Sharding: data-parallel over the batch dim B=8, one graph per NeuronCore.
Each core computes Y[b]^T = (Wc^T @ h[b]^T) as a [256,256] x [256,2048]
matmul with Wc chunks stationary on the PE. Inputs are host-transposed to
[F_IN, N] so every DMA is fully contiguous.
"""

import numpy as np

import concourse.bass as bass
import concourse.mybir as mybir
import concourse.tile as tile
from concourse import bacc
from concourse.bass_utils import run_bass_kernel_spmd

B = 8          # graphs == cores
N = 2048       # nodes per graph
F_IN = 256     # input features (= contraction dim K)
F_OUT = 256    # num_heads * d_head
P = 128        # SBUF/PSUM partitions
NTILE = 512    # PSUM bank free-dim (fp32)

KC = F_IN // P     # 2 contraction chunks
MC = F_OUT // P    # 2 output-feature chunks
NC_ = N // NTILE   # 4 node chunks

# PE matmul dtype: float32 (exact, 4 cycles/row), float32r (reduced-precision
# single pass, 1 cycle/row at N=512), bfloat16 (1 cycle/row, half DMA).
MATMUL_DTYPE = "float32r"

_module_cache = {}

# test.py reads this after calling kernel() to get profile/exec-time info.
LAST_RESULTS = None


def _build_module(mm_dtype: str) -> bass.Bass:
    if mm_dtype == "bfloat16":
        in_dt = mybir.dt.bfloat16
    elif mm_dtype == "float32r":
        in_dt = mybir.dt.float32r
    else:
        in_dt = mybir.dt.float32

    nc = bacc.Bacc(None, target_bir_lowering=False)
    xt = nc.dram_tensor("xt", [F_IN, N], in_dt, kind="ExternalInput")
    wc = nc.dram_tensor("wc", [F_IN, F_OUT], in_dt, kind="ExternalInput")
    yt = nc.dram_tensor("yt", [F_OUT, N], mybir.dt.float32, kind="ExternalOutput")

    XSPLIT = 2  # halves of the node dim per x DMA (earlier matmul start)
    XW = N // XSPLIT

    with tile.TileContext(nc) as tc:
        with (
            tc.tile_pool(name="wpool", bufs=1) as wpool,
            tc.tile_pool(name="xpool", bufs=1) as xpool,
            tc.tile_pool(name="ypool", bufs=1) as ypool,
            tc.tile_pool(name="pspool", bufs=1, space="PSUM") as pspool,
        ):
            w_tiles = [
                wpool.tile([P, F_OUT], in_dt, name=f"w{k}", tag=f"w{k}")
                for k in range(KC)
            ]
            # x[k][j]: contraction chunk k, node half j
            x_tiles = [
                [
                    xpool.tile([P, XW], in_dt, name=f"x{k}_{j}", tag=f"x{k}_{j}")
                    for j in range(XSPLIT)
                ]
                for k in range(KC)
            ]
            for k in range(KC):
                nc.sync.dma_start(w_tiles[k][:], wc[k * P : (k + 1) * P, :])
            # Load k-chunks of the first node-half before the second half so
            # compute can begin as soon as w0+x00 land.
            for j in range(XSPLIT):
                for k in range(KC):
                    nc.sync.dma_start(
                        x_tiles[k][j][:], xt[k * P : (k + 1) * P, j * XW : (j + 1) * XW]
                    )

            for m in range(MC):
                ps = [
                    pspool.tile(
                        [P, NTILE], mybir.dt.float32, name=f"ps{m}_{n}", tag=f"ps{m}_{n}"
                    )
                    for n in range(NC_)
                ]
                y_sb = ypool.tile([P, N], mybir.dt.float32, name=f"y{m}", tag=f"y{m}")
                for n in range(NC_):
                    j = n // (NC_ // XSPLIT)
                    noff = (n % (NC_ // XSPLIT)) * NTILE
                    for k in range(KC):
                        nc.tensor.matmul(
                            ps[n][:],
                            w_tiles[k][:, m * P : (m + 1) * P],
                            x_tiles[k][j][:, noff : noff + NTILE],
                            start=(k == 0),
                            stop=(k == KC - 1),
                        )
                    # Alternate PSUM->SBUF eviction between DVE and ACT so
                    # neither engine's copy chain gates the out-DMA.
                    dst = y_sb[:, n * NTILE : (n + 1) * NTILE]
                    if n % 2 == 0:
                        nc.vector.tensor_copy(dst, ps[n][:])
                    else:
                        nc.scalar.copy(dst, ps[n][:])
                # Two half-row out-DMAs so the first can fly while the second
                # half's copies finish.
                for j in range(XSPLIT):
                    nc.sync.dma_start(
                        yt[m * P : (m + 1) * P, j * XW : (j + 1) * XW],
                        y_sb[:, j * XW : (j + 1) * XW],
                    )
    nc.compile()
    return nc


def _get_module() -> bass.Bass:
    if MATMUL_DTYPE not in _module_cache:
        _module_cache[MATMUL_DTYPE] = _build_module(MATMUL_DTYPE)
    return _module_cache[MATMUL_DTYPE]


def kernel(h: np.ndarray, adj: np.ndarray, W: np.ndarray, **_unused) -> np.ndarray:
    global LAST_RESULTS
    h = np.asarray(h, dtype=np.float32)
    W = np.asarray(W, dtype=np.float32)
    # Wc[f, head*64+d] = W[head, f, d]
    wc = np.ascontiguousarray(W.transpose(1, 0, 2).reshape(F_IN, F_OUT))

    if MATMUL_DTYPE == "bfloat16":
        import ml_dtypes

        cast = lambda a: np.ascontiguousarray(a.astype(ml_dtypes.bfloat16))
    else:
        cast = np.ascontiguousarray

    wc_in = cast(wc)
    in_maps = [{"xt": cast(h[b].T), "wc": wc_in} for b in range(B)]
    nc = _get_module()
    res = run_bass_kernel_spmd(nc, in_maps, core_ids=list(range(B)))
    LAST_RESULTS = res

    out = np.empty((B, N, F_OUT), dtype=np.float32)
    for b in range(B):
        out[b] = res.results[b]["yt"].T
    return out
